# revision 9
# baseline (speedup 1.0000x reference)
"""Trainium2 Bass kernel: 3D-window sparse multi-head attention.

Full op: out = SDPA(hid@Wq, hid@Wk, hid@Wv; 3D local window mask) @ Wo + bo
Shapes: hid [1, 2048, 1024], 16 heads x 64, grid (8 frames, 16, 16), window (3, 5, 5).

Sharding: head-parallel. Each of the 8 cores computes 2 heads end-to-end
(QKV projection slices, windowed attention, Wo row-slice projection) and
writes a full-shape fp32 partial; the host sums the 8 partials and adds bo.

Per-core schedule (single NEFF, fully unrolled, Tile-scheduled):
  phase 1: q,k projections interleaved kc-outer (PE saturates while hidT
           streams in), then v in [s, hd] layout with an appended ones
           column for the softmax denominator.
  phase 2: per frame f: for each head, scoresT blocks [kv=128, q=256]
           (kv band = frames f-1..f+1), exp on ACT (no max-subtraction;
           scores are O(5)), multiplicative (h,w) window mask on DVE,
           PV matmul accumulating [65, 256] (row 64 = denominator),
           reciprocal straight from PSUM, normalization via a broadcast
           matmul + PSUM*PSUM multiply, then this frame's two s-chunks of
           the output projection and their DMA — so out-DMA streams
           throughout instead of serializing at the tail.
"""

import numpy as np

import concourse.bass as bass
import concourse.mybir as mybir
import concourse.tile as tile
from concourse import bacc
from concourse.bass import ds, ts
from concourse.bass_utils import run_bass_kernel_spmd

S, D, NH, HD = 2048, 1024, 16, 64
NCORES = 8
HPC = NH // NCORES          # heads per core = 2
F, GH, GW = 8, 16, 16       # frames, height, width (S = F*GH*GW)
WF, WH, WW = 3, 5, 5        # window sizes
T = GH * GW                 # tokens per frame = 256
P = 128
KC = D // P                 # 8 contraction chunks
SC = S // P                 # 16 seq chunks of 128
NQ = S // 512               # 4 free chunks of 512
VW = 2 * 66                 # v1 row width: [h0 64 | ones 2 | h1 64 | ones 2]
F16 = mybir.dt.float16
F32 = mybir.dt.float32
EXP = mybir.ActivationFunctionType.Exp
MUL = mybir.AluOpType.mult

_nc_cache = {}


def build_nc(debug=False):
    key = bool(debug)
    if key in _nc_cache:
        return _nc_cache[key]
    nc = bacc.Bacc(None, target_bir_lowering=False, debug=False)

    hidt = nc.dram_tensor("hidt", [P, KC, S], F16, kind="ExternalInput")
    wq = nc.dram_tensor("wq", [P, KC, P], F16, kind="ExternalInput")
    wk = nc.dram_tensor("wk", [P, KC, P], F16, kind="ExternalInput")
    wv = nc.dram_tensor("wv", [P, KC, P], F16, kind="ExternalInput")
    wo = nc.dram_tensor("wo", [P, D], F16, kind="ExternalInput")
    m01 = nc.dram_tensor("m01", [P, 2, T], F16, kind="ExternalInput")
    emat = nc.dram_tensor("emat", [P, P], F16, kind="ExternalInput")
    out = nc.dram_tensor("out", [SC, P, D], F32, kind="ExternalOutput")
    dbg = {}
    if debug:
        dbg["qT"] = nc.dram_tensor("dbg_qt", [P, S], F16, kind="ExternalOutput")
        dbg["kT"] = nc.dram_tensor("dbg_kt", [P, S], F16, kind="ExternalOutput")
        dbg["v1"] = nc.dram_tensor("dbg_v1", [P, SC, VW], F16, kind="ExternalOutput")
        dbg["rp"] = nc.dram_tensor("dbg_rp", [P, S], F16, kind="ExternalOutput")
        dbg["oTn"] = nc.dram_tensor("dbg_otn", [P, S], F16, kind="ExternalOutput")

    with tile.TileContext(nc) as tc:
        with (
            tc.tile_pool(name="const", bufs=1) as cpool,
            tc.tile_pool(name="qk", bufs=1) as qkpool,
            tc.tile_pool(name="attn", bufs=4) as apool,
            tc.tile_pool(name="acc", bufs=1) as accpool,
            tc.tile_pool(name="ostage", bufs=4) as opool,
        ):
            # ---- constant loads: small weights first so PE starts ASAP ----
            wq_sb = cpool.tile([P, KC, P], F16, tag="wq")
            nc.sync.dma_start(wq_sb[:], wq[:])
            wk_sb = cpool.tile([P, KC, P], F16, tag="wk")
            nc.sync.dma_start(wk_sb[:], wk[:])
            wv_sb = cpool.tile([P, KC, P], F16, tag="wv")
            nc.sync.dma_start(wv_sb[:], wv[:])
            wo_sb = cpool.tile([P, D], F16, tag="wo")
            nc.sync.dma_start(wo_sb[:], wo[:])
            m01_sb = cpool.tile([P, 2, T], F16, tag="m01")
            nc.sync.dma_start(m01_sb[:], m01[:])
            emat_sb = cpool.tile([P, P], F16, tag="emat")
            nc.sync.dma_start(emat_sb[:], emat[:])
            hidT_sb = cpool.tile([P, KC, S], F16, tag="hidT")
            for kc in range(KC):
                nc.sync.dma_start(hidT_sb[:, kc, :], hidt[:, kc, :])

            qT_sb = qkpool.tile([P, S], F16, tag="qT")
            kT_sb = qkpool.tile([P, S], F16, tag="kT")
            v1_sb = qkpool.tile([P, SC, VW], F16, tag="v1")
            oTn_sb = accpool.tile([P, S], F16, tag="oTn")
            rp_sb = accpool.tile([P, S], F16, tag="rp")

            # ones columns for the PV denominator row; zero the reciprocal
            # staging tile (its never-written rows feed a K=32 matmul)
            nc.gpsimd.memset(rp_sb[:], 0.0)
            nc.vector.memset(v1_sb[:, :, HD : HD + 2], 1.0)
            nc.vector.memset(v1_sb[:, :, 66 + HD : 66 + HD + 2], 1.0)

            # ---- phase 1a: q,k projections (kc-outer: overlap hidT DMA) ----
            with tc.tile_pool(name="pqk", bufs=8, space="PSUM") as pqk:
                psqs = [pqk.tile([P, 512], F32, tag="pqk", name=f"psq{i}")
                        for i in range(NQ)]
                psks = [pqk.tile([P, 512], F32, tag="pqk", name=f"psk{i}")
                        for i in range(NQ)]
                for kc in range(KC):
                    for nch in range(NQ):
                        nc.tensor.matmul(
                            psqs[nch][:], wq_sb[:, kc, :],
                            hidT_sb[:, kc, ts(nch, 512)],
                            start=(kc == 0), stop=(kc == KC - 1),
                        )
                        nc.tensor.matmul(
                            psks[nch][:], wk_sb[:, kc, :],
                            hidT_sb[:, kc, ts(nch, 512)],
                            start=(kc == 0), stop=(kc == KC - 1),
                        )
                for nch in range(NQ):
                    nc.scalar.copy(qT_sb[:, ts(nch, 512)], psqs[nch][:])
                    nc.vector.tensor_copy(kT_sb[:, ts(nch, 512)], psks[nch][:])

            # ---- phase 1b: v in [s, hd] layout (both heads + ones cols) ----
            with tc.tile_pool(name="pv", bufs=3, space="PSUM") as pvp:
                for sc in range(SC):
                    psv = pvp.tile([P, P], F32, tag="psv")
                    for kc in range(KC):
                        nc.tensor.matmul(
                            psv[:], hidT_sb[:, kc, ts(sc, P)], wv_sb[:, kc, :],
                            start=(kc == 0), stop=(kc == KC - 1),
                        )
                    # one strided copy: psv [p, 2, 64] -> v1 cols {0:64, 66:130}
                    dst = v1_sb[:, sc, :].rearrange(
                        "p (two c) -> p two c", two=2
                    )[:, :, 0:HD]
                    src = psv[:].rearrange("p (two c) -> p two c", two=2)
                    nc.vector.tensor_copy(dst, src)

            # ---- phase 2: per-frame attention + normalize + O-proj + DMA ----
            with (
                tc.tile_pool(name="psS", bufs=4, space="PSUM") as pssp,
                tc.tile_pool(name="pso", bufs=1, space="PSUM") as psop,
                tc.tile_pool(name="pbb", bufs=1, space="PSUM") as pbbp,
                tc.tile_pool(name="pO", bufs=2, space="PSUM") as pOp,
                nc.allow_low_precision("softmax reciprocal in fp16"),
            ):
                for f in range(F):
                    lo, hi = max(0, f - 1), min(F - 1, f + 1)
                    nb = hi - lo + 1                     # 2-chunk batches
                    fs = ds(f * T, T)
                    qs = ds(f * T, T)
                    pms = {}
                    # scores + exp + mask for both heads (pipelines on psS)
                    for h in range(HPC):
                        hr = ds(h * HD, HD)
                        for b in range(nb):
                            psS = pssp.tile([P, 2, T], F32, tag="psS")
                            for j in range(2):
                                sckv = 2 * lo + 2 * b + j
                                nc.tensor.matmul(
                                    psS[:, j, :],
                                    kT_sb[hr, ds(sckv * P, P)],
                                    qT_sb[hr, qs],
                                    start=True, stop=True,
                                )
                            et = apool.tile([P, 2, T], F16, tag="et")
                            nc.scalar.activation(et[:], psS[:], EXP)
                            pm = apool.tile([P, 2, T], F16, tag="pm", bufs=6)
                            nc.vector.tensor_tensor(pm[:], et[:], m01_sb[:], MUL)
                            pms[(h, b)] = pm
                    # PV + normalize per head
                    for h in range(HPC):
                        pso = psop.tile([66, T], F32, tag="pso")
                        for b in range(nb):
                            pm = pms[(h, b)]
                            for j in range(2):
                                sckv = 2 * lo + 2 * b + j
                                nc.tensor.matmul(
                                    pso[0 : HD + 1, :],
                                    v1_sb[:, sckv, h * 66 : h * 66 + HD + 1],
                                    pm[:, j, :],
                                    start=(b == 0 and j == 0),
                                    stop=(b == nb - 1 and j == 1),
                                )
                        # head h's reciprocal denominator -> rp row 64*h
                        nc.vector.reciprocal(
                            rp_sb[HD * h : HD * h + 1, fs], pso[HD : HD + 1, :]
                        )
                        pbb = pbbp.tile([HD, T], F32, tag="pbb")
                        nc.tensor.matmul(
                            pbb[:],
                            emat_sb[HD * h : HD * h + 32, ds(h * HD, HD)],
                            rp_sb[HD * h : HD * h + 32, fs],
                            start=True, stop=True,
                        )
                        # DVE may read only one PSUM operand: stage pbb in SBUF
                        pbs = apool.tile([HD, T], F16, tag="pbs", bufs=2)
                        nc.scalar.copy(pbs[:], pbb[:])
                        nc.vector.tensor_tensor(
                            oTn_sb[ds(h * HD, HD), fs], pso[0:HD, :], pbs[:], MUL
                        )
                    # output projection for this frame's two s-chunks
                    for j in range(2):
                        sc = 2 * f + j
                        for n2 in range(2):
                            pO = pOp.tile([P, 512], F32, tag="pO")
                            nc.tensor.matmul(
                                pO[:], oTn_sb[:, ts(sc, P)],
                                wo_sb[:, ts(n2, 512)],
                                start=True, stop=True,
                            )
                            ob = opool.tile([P, 512], F32, tag="ob")
                            if n2 == 0:
                                nc.vector.tensor_copy(ob[:], pO[:])
                            else:
                                nc.scalar.copy(ob[:], pO[:])
                            nc.sync.dma_start(out[sc, :, ts(n2, 512)], ob[:])

            if debug:
                nc.sync.dma_start(dbg["qT"][:], qT_sb[:])
                nc.sync.dma_start(dbg["kT"][:], kT_sb[:])
                nc.sync.dma_start(dbg["v1"][:], v1_sb[:])
                nc.sync.dma_start(dbg["rp"][:], rp_sb[:])
                nc.sync.dma_start(dbg["oTn"][:], oTn_sb[:])

    nc.compile()
    _nc_cache[key] = nc
    return nc


def make_in_maps(hidden_states, Wq, Wk, Wv, Wo):
    """Host-side shard + repack of full inputs into per-core input maps."""
    hid = np.asarray(hidden_states, np.float32).reshape(S, D)
    # hidT packed [ki, ko, s] with d = ko*128 + ki
    hidT_pk = np.ascontiguousarray(
        hid.T.reshape(KC, P, S).transpose(1, 0, 2)
    ).astype(np.float16)

    scale = 1.0 / np.sqrt(HD)
    Wq_s = np.asarray(Wq, np.float32) * scale
    Wk_ = np.asarray(Wk, np.float32)
    Wv_ = np.asarray(Wv, np.float32)
    Wo_ = np.asarray(Wo, np.float32)

    def pack_w(W, c):
        Wc = W[:, c * HPC * HD : (c + 1) * HPC * HD]  # [D, 128]
        return np.ascontiguousarray(
            Wc.reshape(KC, P, HPC * HD).transpose(1, 0, 2)
        ).astype(np.float16)

    # (h, w) window mask, 0/1, [256, 256] (symmetric) packed to [p, c, q]
    idx = np.arange(T)
    hh, ww = idx // GW, idx % GW
    m = (np.abs(hh[:, None] - hh[None, :]) <= WH // 2) & (
        np.abs(ww[:, None] - ww[None, :]) <= WW // 2
    )
    m01_pk = np.ascontiguousarray(
        m.astype(np.float16).reshape(2, P, T).transpose(1, 0, 2)
    )

    # broadcast matrix: head h's reciprocal lives on rp row 64*h; the
    # pbb matmul for head h uses lhsT = emat[64h:64h+32, 64h:64h+64]
    emat = np.zeros((P, P), np.float16)
    emat[0, 0:HD] = 1.0
    emat[HD, HD : 2 * HD] = 1.0

    in_maps = []
    for c in range(NCORES):
        in_maps.append(
            dict(
                hidt=hidT_pk,
                wq=pack_w(Wq_s, c),
                wk=pack_w(Wk_, c),
                wv=pack_w(Wv_, c),
                wo=Wo_[c * HPC * HD : (c + 1) * HPC * HD, :].astype(np.float16),
                m01=m01_pk,
                emat=emat,
            )
        )
    return in_maps


def kernel(
    hidden_states,
    Wq,
    Wk,
    Wv,
    Wo,
    bo,
    frames=F,
    height=GH,
    width=GW,
    wf=WF,
    wh=WH,
    ww=WW,
):
    assert (int(frames), int(height), int(width)) == (F, GH, GW)
    assert (int(wf), int(wh), int(ww)) == (WF, WH, WW)
    in_maps = make_in_maps(hidden_states, Wq, Wk, Wv, Wo)
    nc = build_nc(debug=False)
    res = run_bass_kernel_spmd(nc, in_maps, core_ids=list(range(NCORES)))
    acc = np.zeros((S, D), np.float32)
    for r in res.results:
        acc += r["out"].reshape(S, D)
    acc += np.asarray(bo, np.float32)[None, :]
    return acc.reshape(1, S, D)


# revision 16
# speedup vs baseline: 8.4049x; 8.4049x over previous
"""Trainium2 Bass kernel: 3D-window sparse multi-head attention.

Full op: out = SDPA(hid@Wq, hid@Wk, hid@Wv; 3D local window mask) @ Wo + bo
Shapes: hid [1, 2048, 1024], 16 heads x 64, grid (8 frames, 16, 16), window (3, 5, 5).

Sharding: head-parallel. Each of the 8 cores computes 2 heads end-to-end
(QKV projection slices, windowed attention, Wo row-slice projection) and
writes a full-shape fp32 partial; the host sums the 8 partials and adds bo.

Per-core schedule (single NEFF, fully unrolled, Tile-scheduled):
  phase 1: q,k projections interleaved kc-outer (PE saturates while hidT
           streams in), then v in [s, hd] layout with an appended ones
           column for the softmax denominator.
  phase 2: per frame f: for each head, scoresT blocks [kv=128, q=256]
           (kv band = frames f-1..f+1), exp on ACT (no max-subtraction;
           scores are O(5)), multiplicative (h,w) window mask on DVE,
           PV matmul accumulating [65, 256] (row 64 = denominator),
           reciprocal straight from PSUM, normalization via a broadcast
           matmul + PSUM*PSUM multiply, then this frame's two s-chunks of
           the output projection and their DMA — so out-DMA streams
           throughout instead of serializing at the tail.
"""

import numpy as np

import concourse.bass as bass
import concourse.mybir as mybir
import concourse.tile as tile
from concourse import bacc
from concourse.bass import ds, ts
from concourse.bass_utils import run_bass_kernel_spmd

S, D, NH, HD = 2048, 1024, 16, 64
NCORES = 8
HPC = NH // NCORES          # heads per core = 2
F, GH, GW = 8, 16, 16       # frames, height, width (S = F*GH*GW)
WF, WH, WW = 3, 5, 5        # window sizes
T = GH * GW                 # tokens per frame = 256
P = 128
KC = D // P                 # 8 contraction chunks
SC = S // P                 # 16 seq chunks of 128
NQ = S // 512               # 4 free chunks of 512
VW = 2 * 66                 # v1 row width: [h0 64 | ones 2 | h1 64 | ones 2]
F16 = mybir.dt.float16
F32 = mybir.dt.float32
EXP = mybir.ActivationFunctionType.Exp
MUL = mybir.AluOpType.mult

_nc_cache = {}


def build_nc(debug=False):
    key = bool(debug)
    if key in _nc_cache:
        return _nc_cache[key]
    nc = bacc.Bacc(None, target_bir_lowering=False, debug=False)

    hidt = nc.dram_tensor("hidt", [P, KC, S], F16, kind="ExternalInput")
    wq = nc.dram_tensor("wq", [P, KC, P], F16, kind="ExternalInput")
    wk = nc.dram_tensor("wk", [P, KC, P], F16, kind="ExternalInput")
    wv = nc.dram_tensor("wv", [P, KC, P], F16, kind="ExternalInput")
    wo = nc.dram_tensor("wo", [P, D], F16, kind="ExternalInput")
    m01 = nc.dram_tensor("m01", [P, 2, T], F16, kind="ExternalInput")
    out = nc.dram_tensor("out", [SC, P, D], F32, kind="ExternalOutput")
    dbg = {}
    if debug:
        dbg["qT"] = nc.dram_tensor("dbg_qt", [P, S], F16, kind="ExternalOutput")
        dbg["kT"] = nc.dram_tensor("dbg_kt", [P, S], F16, kind="ExternalOutput")
        dbg["v1"] = nc.dram_tensor("dbg_v1", [P, SC, VW], F16, kind="ExternalOutput")
        dbg["rp"] = nc.dram_tensor("dbg_rp", [1, 2 * S], F16, kind="ExternalOutput")
        dbg["oTn"] = nc.dram_tensor("dbg_otn", [P, S], F16, kind="ExternalOutput")

    with tile.TileContext(nc) as tc:
        with (
            tc.tile_pool(name="const", bufs=1) as cpool,
            tc.tile_pool(name="qk", bufs=1) as qkpool,
            tc.tile_pool(name="attn", bufs=4) as apool,
            tc.tile_pool(name="acc", bufs=1) as accpool,
            tc.tile_pool(name="ostage", bufs=4) as opool,
        ):
            # ---- constant loads: small weights first so PE starts ASAP ----
            wq_sb = cpool.tile([P, KC, P], F16, tag="wq")
            nc.sync.dma_start(wq_sb[:], wq[:])
            wk_sb = cpool.tile([P, KC, P], F16, tag="wk")
            nc.sync.dma_start(wk_sb[:], wk[:])
            wv_sb = cpool.tile([P, KC, P], F16, tag="wv")
            nc.sync.dma_start(wv_sb[:], wv[:])
            wo_sb = cpool.tile([P, D], F16, tag="wo")
            nc.sync.dma_start(wo_sb[:], wo[:])
            m01_sb = cpool.tile([P, 2, T], F16, tag="m01")
            nc.sync.dma_start(m01_sb[:], m01[:])
            hidT_sb = cpool.tile([P, KC, S], F16, tag="hidT")
            for kc in range(KC):
                nc.sync.dma_start(hidT_sb[:, kc, :], hidt[:, kc, :])

            qT_sb = qkpool.tile([P, S], F16, tag="qT")
            kT_sb = qkpool.tile([P, S], F16, tag="kT")
            v1_sb = qkpool.tile([P, SC, VW], F16, tag="v1")
            oTn_sb = accpool.tile([P, S], F16, tag="oTn")
            # reciprocal denominators, head h at cols [h*S, (h+1)*S) of row 0
            # (gpsimd partition_broadcast requires its source on partition 0)
            rp_sb = accpool.tile([1, HPC * S], F16, tag="rp")

            # ones columns for the PV denominator row
            nc.vector.memset(v1_sb[:, :, HD : HD + 2], 1.0)
            nc.vector.memset(v1_sb[:, :, 66 + HD : 66 + HD + 2], 1.0)

            # ---- phase 1a: q,k projections (kc-outer: overlap hidT DMA) ----
            with tc.tile_pool(name="pqk", bufs=8, space="PSUM") as pqk:
                psqs = [pqk.tile([P, 512], F32, tag="pqk", name=f"psq{i}")
                        for i in range(NQ)]
                psks = [pqk.tile([P, 512], F32, tag="pqk", name=f"psk{i}")
                        for i in range(NQ)]
                for kc in range(KC):
                    for nch in range(NQ):
                        nc.tensor.matmul(
                            psqs[nch][:], wq_sb[:, kc, :],
                            hidT_sb[:, kc, ts(nch, 512)],
                            start=(kc == 0), stop=(kc == KC - 1),
                        )
                        nc.tensor.matmul(
                            psks[nch][:], wk_sb[:, kc, :],
                            hidT_sb[:, kc, ts(nch, 512)],
                            start=(kc == 0), stop=(kc == KC - 1),
                        )
                for nch in range(NQ):
                    nc.scalar.copy(qT_sb[:, ts(nch, 512)], psqs[nch][:])
                    nc.vector.tensor_copy(kT_sb[:, ts(nch, 512)], psks[nch][:])

            # ---- phase 1b: v in [s, hd] layout (both heads + ones cols) ----
            with tc.tile_pool(name="pv", bufs=3, space="PSUM") as pvp:
                for sc in range(SC):
                    psv = pvp.tile([P, P], F32, tag="psv")
                    for kc in range(KC):
                        nc.tensor.matmul(
                            psv[:], hidT_sb[:, kc, ts(sc, P)], wv_sb[:, kc, :],
                            start=(kc == 0), stop=(kc == KC - 1),
                        )
                    # one strided copy: psv [p, 2, 64] -> v1 cols {0:64, 66:130}
                    dst = v1_sb[:, sc, :].rearrange(
                        "p (two c) -> p two c", two=2
                    )[:, :, 0:HD]
                    src = psv[:].rearrange("p (two c) -> p two c", two=2)
                    nc.vector.tensor_copy(dst, src)

            # ---- phase 2: per-frame attention + normalize + O-proj + DMA ----
            with (
                tc.tile_pool(name="psS", bufs=4, space="PSUM") as pssp,
                tc.tile_pool(name="pso", bufs=2, space="PSUM") as psop,
                tc.tile_pool(name="pO", bufs=2, space="PSUM") as pOp,
                nc.allow_low_precision("softmax reciprocal in fp16"),
            ):
                for f in range(F):
                    lo, hi = max(0, f - 1), min(F - 1, f + 1)
                    nb = hi - lo + 1                     # 2-chunk batches
                    fs = ds(f * T, T)
                    pms = {}
                    # scores + exp + mask for both heads (pipelines on psS).
                    # The (h,w) window implies a kv-h band: an even kv chunk
                    # (h 0..7) only reaches q columns 0:160, an odd chunk
                    # (h 8..15) only 96:256 — matmuls touch just those 160
                    # "live" columns; PSUM's per-element has_written bits make
                    # the partial-coverage accumulation exact.
                    for h in range(HPC):
                        hr = ds(h * HD, HD)
                        for b in range(nb):
                            psS = pssp.tile([P, 2, T], F32, tag="psS")
                            for j in range(2):
                                sckv = 2 * lo + 2 * b + j
                                nc.tensor.matmul(
                                    psS[:, j, 96 * j : 96 * j + 160],
                                    kT_sb[hr, ds(sckv * P, P)],
                                    qT_sb[hr, ds(f * T + 96 * j, 160)],
                                    start=True, stop=True,
                                )
                            et = apool.tile([P, 2, T], F16, tag="et")
                            nc.scalar.activation(et[:], psS[:], EXP)
                            pm = apool.tile([P, 2, T], F16, tag="pm", bufs=6)
                            nc.vector.tensor_tensor(pm[:], et[:], m01_sb[:], MUL)
                            pms[(h, b)] = pm
                    # PV + normalize per head
                    for h in range(HPC):
                        pso = psop.tile([66, T], F32, tag="pso")
                        for b in range(nb):
                            pm = pms[(h, b)]
                            for j in range(2):
                                sckv = 2 * lo + 2 * b + j
                                nc.tensor.matmul(
                                    pso[0 : HD + 1, 96 * j : 96 * j + 160],
                                    v1_sb[:, sckv, h * 66 : h * 66 + HD + 1],
                                    pm[:, j, 96 * j : 96 * j + 160],
                                    start=(b == 0 and j == 0),
                                    stop=(b == nb - 1 and j == 1),
                                )
                        # reciprocal of the denominator row, then broadcast it
                        # across 64 partitions on the (otherwise idle) GPSIMD
                        rps = ds(h * S + f * T, T)
                        nc.vector.reciprocal(rp_sb[0:1, rps], pso[HD : HD + 1, :])
                        pbs = apool.tile([HD, T], F16, tag="pbs", bufs=2)
                        nc.gpsimd.partition_broadcast(pbs[:], rp_sb[0:1, rps])
                        nc.vector.tensor_tensor(
                            oTn_sb[ds(h * HD, HD), fs], pso[0:HD, :], pbs[:], MUL
                        )
                    # output projection for this frame's two s-chunks
                    for j in range(2):
                        sc = 2 * f + j
                        for n2 in range(2):
                            pO = pOp.tile([P, 512], F32, tag="pO")
                            nc.tensor.matmul(
                                pO[:], oTn_sb[:, ts(sc, P)],
                                wo_sb[:, ts(n2, 512)],
                                start=True, stop=True,
                            )
                            ob = opool.tile([P, 512], F32, tag="ob")
                            if n2 == 0:
                                nc.vector.tensor_copy(ob[:], pO[:])
                            else:
                                nc.scalar.copy(ob[:], pO[:])
                            nc.sync.dma_start(out[sc, :, ts(n2, 512)], ob[:])

            if debug:
                nc.sync.dma_start(dbg["qT"][:], qT_sb[:])
                nc.sync.dma_start(dbg["kT"][:], kT_sb[:])
                nc.sync.dma_start(dbg["v1"][:], v1_sb[:])
                nc.sync.dma_start(dbg["rp"][:], rp_sb[:])
                nc.sync.dma_start(dbg["oTn"][:], oTn_sb[:])

    nc.compile()
    _nc_cache[key] = nc
    return nc


def make_in_maps(hidden_states, Wq, Wk, Wv, Wo):
    """Host-side shard + repack of full inputs into per-core input maps."""
    hid = np.asarray(hidden_states, np.float32).reshape(S, D)
    # hidT packed [ki, ko, s] with d = ko*128 + ki
    hidT_pk = np.ascontiguousarray(
        hid.T.reshape(KC, P, S).transpose(1, 0, 2)
    ).astype(np.float16)

    scale = 1.0 / np.sqrt(HD)
    Wq_s = np.asarray(Wq, np.float32) * scale
    Wk_ = np.asarray(Wk, np.float32)
    Wv_ = np.asarray(Wv, np.float32)
    Wo_ = np.asarray(Wo, np.float32)

    def pack_w(W, c):
        Wc = W[:, c * HPC * HD : (c + 1) * HPC * HD]  # [D, 128]
        return np.ascontiguousarray(
            Wc.reshape(KC, P, HPC * HD).transpose(1, 0, 2)
        ).astype(np.float16)

    # (h, w) window mask, 0/1, [256, 256] (symmetric) packed to [p, c, q]
    idx = np.arange(T)
    hh, ww = idx // GW, idx % GW
    m = (np.abs(hh[:, None] - hh[None, :]) <= WH // 2) & (
        np.abs(ww[:, None] - ww[None, :]) <= WW // 2
    )
    m01_pk = np.ascontiguousarray(
        m.astype(np.float16).reshape(2, P, T).transpose(1, 0, 2)
    )

    in_maps = []
    for c in range(NCORES):
        in_maps.append(
            dict(
                hidt=hidT_pk,
                wq=pack_w(Wq_s, c),
                wk=pack_w(Wk_, c),
                wv=pack_w(Wv_, c),
                wo=Wo_[c * HPC * HD : (c + 1) * HPC * HD, :].astype(np.float16),
                m01=m01_pk,
            )
        )
    return in_maps


def kernel(
    hidden_states,
    Wq,
    Wk,
    Wv,
    Wo,
    bo,
    frames=F,
    height=GH,
    width=GW,
    wf=WF,
    wh=WH,
    ww=WW,
):
    assert (int(frames), int(height), int(width)) == (F, GH, GW)
    assert (int(wf), int(wh), int(ww)) == (WF, WH, WW)
    in_maps = make_in_maps(hidden_states, Wq, Wk, Wv, Wo)
    nc = build_nc(debug=False)
    res = run_bass_kernel_spmd(nc, in_maps, core_ids=list(range(NCORES)))
    acc = np.zeros((S, D), np.float32)
    for r in res.results:
        acc += r["out"].reshape(S, D)
    acc += np.asarray(bo, np.float32)[None, :]
    return acc.reshape(1, S, D)
